# revision 32
# baseline (speedup 1.0000x reference)
"""Trainium2 Bass kernel for CRF mean-field iteration (nn_CRF).

Math (derived from the reference):
    comp = -I  =>  each iteration is   x <- x0 + w * smooth(softmax(x, C))
    output = log_softmax(x_final, C)
where smooth = per-channel separable 11-tap Gaussian blur over H then W
('same' zero padding, center tap zeroed, per-sample spacing).

v3 strategy (per core, 2 samples in channel-level lockstep):
  - All conv matmuls in bf16 (1 cyc/row on PE; FWL weight loads). Th/Tw
    and x cast to bf16 on the host.
  - x-update rides on the PE: an identity-matmul accumulates x0 into the
    W-conv PSUM group (identity first with start=True, convs accumulate
    on top). PSUM then holds x_new; ACT exp reads it directly and writes
    e = exp(x_new) to SBUF in bf16.
  - Channel sum S also rides on the PE: 16 identity-matmuls accumulate
    e[c] into a PSUM slot (exact fp32 sum). r = 1/S via fast reciprocal
    (PSUM source), cast to bf16, p = e*r in-place at DVE 2x.
  - Two samples interleaved per channel: each sample owns a 1-deep
    [P,3,512] PSUM ring (pA -> pB -> ... -> S), so engine program order
    always has the other sample's independent work available; PE never
    idles long enough for HAM to re-throttle.
  - o1/xf PSUM evacuations split between ACT and DVE by knobs; final
    out = xf - ln(S5) subtraction on GPSIMD (SBUF-only); DMA out per
    channel.
"""

import sys

if "/opt/trn_rl_repo" not in sys.path:
    sys.path.insert(0, "/opt/trn_rl_repo")

from contextlib import ExitStack

import numpy as np

import concourse.bass as bass
import concourse.tile as tile
from concourse import bacc, mybir

F32 = mybir.dt.float32
BF16 = mybir.dt.bfloat16
AF = mybir.ActivationFunctionType

B, C, H, W = 16, 16, 384, 384
N_CORES = 8
BPC = B // N_CORES  # samples per core
N_ITER = 5
FS = 11
HALF = FS // 2  # 5
P = 128
NCH = H // P  # 3 h-chunks
NCW = W // P  # 3 w-chunks

# --- engine assignment knobs ---
# o1 PSUM->SBUF copy: channels with (c % 16) < O1_ACT_CH go to ACT, rest DVE.
O1_ACT_CH = 6
# xf PSUM->SBUF copy on last iteration: c < XF_ACT_CH -> ACT, rest DVE.
XF_ACT_CH = 12
# p = e*r multiply: channels with c >= C - PMUL_POOL_CH go to gpsimd.
PMUL_POOL_CH = 3
# final sub: channels with c >= C - SUB_POOL_CH go to gpsimd, rest DVE.
SUB_POOL_CH = 4
# sample B emission stagger, in chunks (~1 channel each).
DESYNC = 0


def _band(j, n):
    return max(0, P * j - HALF), min(n, P * j + P + HALF)


class SampleCtx:
    """Per-sample tiles and emission state."""

    def __init__(self, tc, pools, b):
        nc = tc.nc
        state, shared, mats, psum = pools
        self.b = b
        self.x0 = state.tile([P, C, NCH, W], BF16, tag=f"x0_{b}")
        self.ep = state.tile([P, C, NCH, W], BF16, tag=f"ep_{b}")
        self.rr = state.tile([P, NCH, W], F32, tag=f"rr_{b}")
        self.rb = state.tile([P, NCH, W], BF16, tag=f"rb_{b}")
        self.lnb = state.tile([P, NCH, W], BF16, tag=f"lnb_{b}")
        self.sacc = state.tile([P, NCH, W], BF16, tag=f"sacc_{b}")
        self.th = mats.tile([P, NCH, H], BF16, tag=f"th_{b}")
        self.tw = mats.tile([P, NCW, W], BF16, tag=f"tw_{b}")
        self.psum = psum  # per-sample pool
        self.pA = None
        self.pB = None


def _crf_kernel(ctx, tc, out_d, x_in, th_in, tw_in, id_in, n_samples, n_iter):
    nc = tc.nc
    assert n_samples == 2

    state = ctx.enter_context(tc.tile_pool(name="state", bufs=1))
    shared = ctx.enter_context(tc.tile_pool(name="shared", bufs=1))
    mats = ctx.enter_context(tc.tile_pool(name="mats", bufs=1))
    o1p = ctx.enter_context(tc.tile_pool(name="o1p", bufs=4))
    outp = ctx.enter_context(tc.tile_pool(name="outp", bufs=4))
    psA = ctx.enter_context(tc.tile_pool(name="psA", bufs=1, space="PSUM"))
    psB = ctx.enter_context(tc.tile_pool(name="psB", bufs=1, space="PSUM"))
    ident = shared.tile([P, P], BF16, tag="ident")
    nc.sync.dma_start(out=ident[:], in_=id_in[:])

    S = [SampleCtx(tc, (state, shared, mats, ps), b)
         for b, ps in zip(range(n_samples), (psA, psB))]

    for s in S:
        b = s.b
        nc.sync.dma_start(
            out=s.x0[:], in_=x_in[b].rearrange("c (j p) w -> p c j w", p=P)
        )
        nc.sync.dma_start(out=s.th[:], in_=th_in[b].rearrange("(j p) n -> p j n", p=P))
        nc.sync.dma_start(out=s.tw[:], in_=tw_in[b].rearrange("(j p) n -> p j n", p=P))

    # --- emission helpers (all take a SampleCtx) ---
    def emit_hconv(s, c):
        s.pA = s.psum.tile([P, NCH, 512], F32, tag=f"ps{s.b}")
        for m in range(NCW):
            for j in range(NCH):
                n0, n1 = _band(j, H)
                nc.tensor.matmul(
                    s.pA[:, m, n0:n1],
                    lhsT=s.ep[:, c, j, m * P : (m + 1) * P],
                    rhs=s.th[:, j, n0:n1],
                    start=(j == 0),
                    stop=(j == NCH - 1),
                )

    def emit_o1(s, c):
        o1 = o1p.tile([P, NCW, H], BF16, tag="o1")
        if c < O1_ACT_CH:
            nc.scalar.copy(out=o1[:], in_=s.pA[:, :, 0:H])
        else:
            nc.vector.tensor_copy(o1[:], s.pA[:, :, 0:H])
        s.o1 = o1

    def emit_wconv(s, c):
        s.pB = s.psum.tile([P, NCH, 512], F32, tag=f"ps{s.b}")
        for m in range(NCH):
            nc.tensor.matmul(
                s.pB[:, m, 0:W],
                lhsT=ident[:],
                rhs=s.x0[:, c, m, :],
                start=True,
                stop=False,
            )
            for j in range(NCW):
                n0, n1 = _band(j, W)
                nc.tensor.matmul(
                    s.pB[:, m, n0:n1],
                    lhsT=s.o1[:, j, m * P : (m + 1) * P],
                    rhs=s.tw[:, j, n0:n1],
                    start=False,
                    stop=(j == NCW - 1),
                )

    def emit_exp_psum(s, c, last):
        nc.scalar.activation(out=s.ep[:, c], in_=s.pB[:, :, 0:W], func=AF.Exp)
        if last:
            if c < XF_ACT_CH:
                nc.scalar.copy(out=s.x0[:, c], in_=s.pB[:, :, 0:W])
            else:
                nc.vector.tensor_copy(s.x0[:, c], s.pB[:, :, 0:W])
            # progressive S5 accumulation (only ln(S5) is needed at the
            # end; avoids the ring-blocked ssum at the final boundary)
            if c == 1:
                nc.vector.tensor_add(s.sacc[:], s.ep[:, 0], s.ep[:, 1])
            elif c >= 2:
                nc.vector.tensor_add(s.sacc[:], s.sacc[:], s.ep[:, c])

    def emit_ssum(s):
        """S = sum_c e[c] via PE identity-matmuls into the PSUM ring."""
        s.pS = s.psum.tile([P, NCH, 512], F32, tag=f"ps{s.b}")
        for c in range(C):
            for j in range(NCH):
                nc.tensor.matmul(
                    s.pS[:, j, 0:W],
                    lhsT=ident[:],
                    rhs=s.ep[:, c, j, :],
                    start=(c == 0),
                    stop=(c == C - 1),
                )

    def emit_recip(s):
        nc.vector.reciprocal_approx_fast(out=s.rr[:], in_=s.pS[:, :, 0:W])
        nc.vector.tensor_copy(s.rb[:], s.rr[:])

    def emit_pmul(s, c):
        eng = nc.gpsimd if c >= C - PMUL_POOL_CH else nc.vector
        eng.tensor_mul(out=s.ep[:, c], in0=s.ep[:, c], in1=s.rb[:])

    def emit_sub_dma(s, c):
        ot = outp.tile([P, NCH, W], BF16, tag="ot")
        eng = nc.gpsimd if c >= C - SUB_POOL_CH else nc.vector
        eng.tensor_sub(ot[:], s.x0[:, c], s.lnb[:])
        nc.sync.dma_start(
            out=out_d[s.b, c].rearrange("(j p) w -> p j w", p=P),
            in_=ot[:],
        )

    # --- build per-sample chunk schedules; each chunk maps pipeline-stage
    # -> emitter. The driver staggers sample B by DESYNC chunks and emits
    # stage-by-stage across samples so each engine's in-order stream has
    # the other sample's ready work ahead of any stalling op. ---
    def build_chunks(s):
        from functools import partial

        def pro_exp(s, c):
            # two channels per op: amortizes the ACT pipe/decode overhead
            nc.scalar.activation(
                out=s.ep[:, c : c + 2], in_=s.x0[:, c : c + 2], func=AF.Exp
            )

        def pmul_hconv(s, c):
            emit_pmul(s, c)
            emit_hconv(s, c)

        def ln_final(s):
            nc.scalar.activation(out=s.lnb[:], in_=s.sacc[:], func=AF.Ln)

        chunks = []
        for c in range(0, C, 2):
            chunks.append({3: partial(pro_exp, s, c)})
        chunks.append({0: partial(emit_ssum, s)})
        chunks.append({0: partial(emit_recip, s)})
        for it in range(n_iter):
            last = it == n_iter - 1
            for c in range(C):
                chunks.append({
                    0: partial(pmul_hconv, s, c),
                    1: partial(emit_o1, s, c),
                    2: partial(emit_wconv, s, c),
                    3: partial(emit_exp_psum, s, c, last),
                })
            if not last:
                chunks.append({0: partial(emit_ssum, s)})
                chunks.append({0: partial(emit_recip, s)})
            else:
                chunks.append({0: partial(ln_final, s)})
        for c in range(C):
            chunks.append({1: partial(emit_sub_dma, s, c)})
        return chunks

    scheds = [(build_chunks(s), b * DESYNC) for s, b in zip(S, range(n_samples))]
    n_slots = max(len(ch) + off for ch, off in scheds)
    for t in range(n_slots):
        for stage in range(4):
            for ch, off in scheds:
                i = t - off
                if 0 <= i < len(ch) and stage in ch[i]:
                    ch[i][stage]()


def build_nc(n_samples=BPC, n_iter=N_ITER, full_j0=False):
    nc = bacc.Bacc()
    x_in = nc.dram_tensor("x", [n_samples, C, H, W], BF16, kind="ExternalInput")
    th_in = nc.dram_tensor("th", [n_samples, H, H], BF16, kind="ExternalInput")
    tw_in = nc.dram_tensor("tw", [n_samples, W, W], BF16, kind="ExternalInput")
    id_in = nc.dram_tensor("ident", [P, P], BF16, kind="ExternalInput")
    out_d = nc.dram_tensor("out", [n_samples, C, H, W], BF16, kind="ExternalOutput")
    with tile.TileContext(nc) as tc:
        with ExitStack() as ctx:
            _crf_kernel(ctx, tc, out_d, x_in, th_in, tw_in, id_in, n_samples, n_iter)
    nc.finalize()
    return nc


def make_toeplitz(spacing, inv_theta, size, weight=1.0):
    d = spacing * np.arange(-(FS // 2), FS // 2 + 1, dtype=np.float32)
    k = np.exp(-((d * inv_theta) ** 2) / 2.0).astype(np.float32)
    k[FS // 2] = 0.0
    t = np.zeros((size, size), dtype=np.float32)
    for tap in range(FS):
        off = tap - FS // 2
        idx = np.arange(max(0, -off), min(size, size - off))
        t[idx + off, idx] = k[tap]
    return (t * weight).astype(np.float32)


def to_bf16(a):
    import ml_dtypes

    return np.asarray(a, dtype=np.float32).astype(ml_dtypes.bfloat16)


def host_prep(x, spatial_spacings, smoothness_weight, inv_smoothness_theta):
    w = float(np.asarray(smoothness_weight))
    th = np.stack(
        [
            make_toeplitz(float(spatial_spacings[b, 0]), float(inv_smoothness_theta[0]), H)
            for b in range(x.shape[0])
        ]
    )
    tw = np.stack(
        [
            make_toeplitz(
                float(spatial_spacings[b, 1]), float(inv_smoothness_theta[1]), W, weight=w
            )
            for b in range(x.shape[0])
        ]
    )
    return to_bf16(th), to_bf16(tw)


_NC_CACHE = {}


def kernel(x, spatial_spacings, smoothness_weight, inv_smoothness_theta):
    from concourse.bass_utils import run_bass_kernel_spmd

    x = to_bf16(x)
    spatial_spacings = np.asarray(spatial_spacings, dtype=np.float32)
    th, tw = host_prep(x, spatial_spacings, smoothness_weight, inv_smoothness_theta)
    ident = to_bf16(np.eye(P, dtype=np.float32))

    key = (BPC, N_ITER)
    if key not in _NC_CACHE:
        _NC_CACHE[key] = build_nc(BPC, N_ITER)
    nc = _NC_CACHE[key]

    core_ids = list(range(N_CORES))
    in_maps = []
    for i in core_ids:
        sl = slice(i * BPC, (i + 1) * BPC)
        in_maps.append({"x": x[sl], "th": th[sl], "tw": tw[sl], "ident": ident})
    res = run_bass_kernel_spmd(nc, in_maps, core_ids)
    out = np.concatenate([res.results[i]["out"] for i in core_ids], axis=0)
    return out.astype(np.float32)


if __name__ == "__main__":
    rng = np.random.default_rng(0)
    x = rng.standard_normal((B, C, H, W), dtype=np.float32)
    out = kernel(
        x,
        np.ones((B, 2), np.float32),
        np.float32(1.0),
        np.ones((2,), np.float32),
    )
    print(out.shape, out.dtype)


# revision 33
# speedup vs baseline: 1.1965x; 1.1965x over previous
"""Trainium2 Bass kernel for CRF mean-field iteration (nn_CRF).

Math (derived from the reference):
    comp = -I  =>  each iteration is   x <- x0 + w * smooth(softmax(x, C))
    output = log_softmax(x_final, C)
where smooth = per-channel separable 11-tap Gaussian blur over H then W
('same' zero padding, center tap zeroed, per-sample spacing).

v3 strategy (per core, 2 samples in channel-level lockstep):
  - All conv matmuls in bf16 (1 cyc/row on PE; FWL weight loads). Th/Tw
    and x cast to bf16 on the host.
  - x-update rides on the PE: an identity-matmul accumulates x0 into the
    W-conv PSUM group (identity first with start=True, convs accumulate
    on top). PSUM then holds x_new; ACT exp reads it directly and writes
    e = exp(x_new) to SBUF in bf16.
  - Channel sum S also rides on the PE: 16 identity-matmuls accumulate
    e[c] into a PSUM slot (exact fp32 sum). r = 1/S via fast reciprocal
    (PSUM source), cast to bf16, p = e*r in-place at DVE 2x.
  - Two samples interleaved per channel: each sample owns a 1-deep
    [P,3,512] PSUM ring (pA -> pB -> ... -> S), so engine program order
    always has the other sample's independent work available; PE never
    idles long enough for HAM to re-throttle.
  - o1/xf PSUM evacuations split between ACT and DVE by knobs; final
    out = xf - ln(S5) subtraction on GPSIMD (SBUF-only); DMA out per
    channel.
"""

import sys

if "/opt/trn_rl_repo" not in sys.path:
    sys.path.insert(0, "/opt/trn_rl_repo")

from contextlib import ExitStack

import numpy as np

import concourse.bass as bass
import concourse.tile as tile
from concourse import bacc, mybir

F32 = mybir.dt.float32
BF16 = mybir.dt.bfloat16
AF = mybir.ActivationFunctionType

B, C, H, W = 16, 16, 384, 384
N_CORES = 8
BPC = B // N_CORES  # samples per core
N_ITER = 5
FS = 11
HALF = FS // 2  # 5
P = 128
NCH = H // P  # 3 h-chunks
NCW = W // P  # 3 w-chunks

# --- engine assignment knobs ---
# o1 PSUM->SBUF copy: channels with (c % 16) < O1_ACT_CH go to ACT, rest DVE.
O1_ACT_CH = 6
# xf PSUM->SBUF copy on last iteration: c < XF_ACT_CH -> ACT, rest DVE.
XF_ACT_CH = 8
# p = e*r multiply: channels with c >= C - PMUL_POOL_CH go to gpsimd.
PMUL_POOL_CH = 3
# final sub: channels with c >= C - SUB_POOL_CH go to gpsimd, rest DVE.
SUB_POOL_CH = 4
# sample B emission stagger, in chunks (~1 channel each).
DESYNC = 0


def _band(j, n):
    return max(0, P * j - HALF), min(n, P * j + P + HALF)


class SampleCtx:
    """Per-sample tiles and emission state."""

    def __init__(self, tc, pools, b):
        nc = tc.nc
        state, shared, mats, psum = pools
        self.b = b
        self.x0 = state.tile([P, C, NCH, W], BF16, tag=f"x0_{b}")
        self.ep = state.tile([P, C, NCH, W], BF16, tag=f"ep_{b}")
        self.rr = state.tile([P, NCH, W], F32, tag=f"rr_{b}")
        self.rb = state.tile([P, NCH, W], BF16, tag=f"rb_{b}")
        self.lnb = state.tile([P, NCH, W], BF16, tag=f"lnb_{b}")
        self.sacc = state.tile([P, NCH, W], BF16, tag=f"sacc_{b}")
        self.th = mats.tile([P, NCH, H], BF16, tag=f"th_{b}")
        self.tw = mats.tile([P, NCW, W], BF16, tag=f"tw_{b}")
        self.psum = psum  # per-sample pool
        self.pA = None
        self.pB = None


def _crf_kernel(ctx, tc, out_d, x_in, th_in, tw_in, id_in, n_samples, n_iter):
    nc = tc.nc
    assert n_samples == 2

    state = ctx.enter_context(tc.tile_pool(name="state", bufs=1))
    shared = ctx.enter_context(tc.tile_pool(name="shared", bufs=1))
    mats = ctx.enter_context(tc.tile_pool(name="mats", bufs=1))
    o1p = ctx.enter_context(tc.tile_pool(name="o1p", bufs=4))
    outp = ctx.enter_context(tc.tile_pool(name="outp", bufs=4))
    psA = ctx.enter_context(tc.tile_pool(name="psA", bufs=1, space="PSUM"))
    psB = ctx.enter_context(tc.tile_pool(name="psB", bufs=1, space="PSUM"))
    ident = shared.tile([P, P], BF16, tag="ident")
    nc.sync.dma_start(out=ident[:], in_=id_in[:])

    S = [SampleCtx(tc, (state, shared, mats, ps), b)
         for b, ps in zip(range(n_samples), (psA, psB))]

    for s in S:
        b = s.b
        nc.sync.dma_start(
            out=s.x0[:], in_=x_in[b].rearrange("c (j p) w -> p c j w", p=P)
        )
        nc.sync.dma_start(out=s.th[:], in_=th_in[b].rearrange("(j p) n -> p j n", p=P))
        nc.sync.dma_start(out=s.tw[:], in_=tw_in[b].rearrange("(j p) n -> p j n", p=P))

    # --- emission helpers (all take a SampleCtx) ---
    def emit_hconv(s, c):
        s.pA = s.psum.tile([P, NCH, 512], F32, tag=f"ps{s.b}")
        for m in range(NCW):
            for j in range(NCH):
                n0, n1 = _band(j, H)
                nc.tensor.matmul(
                    s.pA[:, m, n0:n1],
                    lhsT=s.ep[:, c, j, m * P : (m + 1) * P],
                    rhs=s.th[:, j, n0:n1],
                    start=(j == 0),
                    stop=(j == NCH - 1),
                )

    def emit_o1(s, c):
        o1 = o1p.tile([P, NCW, H], BF16, tag="o1")
        if c < O1_ACT_CH:
            nc.scalar.copy(out=o1[:], in_=s.pA[:, :, 0:H])
        else:
            nc.vector.tensor_copy(o1[:], s.pA[:, :, 0:H])
        s.o1 = o1

    def emit_wconv(s, c):
        s.pB = s.psum.tile([P, NCH, 512], F32, tag=f"ps{s.b}")
        for m in range(NCH):
            nc.tensor.matmul(
                s.pB[:, m, 0:W],
                lhsT=ident[:],
                rhs=s.x0[:, c, m, :],
                start=True,
                stop=False,
            )
            for j in range(NCW):
                n0, n1 = _band(j, W)
                nc.tensor.matmul(
                    s.pB[:, m, n0:n1],
                    lhsT=s.o1[:, j, m * P : (m + 1) * P],
                    rhs=s.tw[:, j, n0:n1],
                    start=False,
                    stop=(j == NCW - 1),
                )

    def emit_exp_psum(s, c, last):
        nc.scalar.activation(out=s.ep[:, c], in_=s.pB[:, :, 0:W], func=AF.Exp)
        if last:
            if c < XF_ACT_CH:
                nc.scalar.copy(out=s.x0[:, c], in_=s.pB[:, :, 0:W])
            else:
                nc.vector.tensor_copy(s.x0[:, c], s.pB[:, :, 0:W])
            # progressive S5 accumulation (only ln(S5) is needed at the
            # end; avoids the ring-blocked ssum at the final boundary)
            if c == 1:
                nc.vector.tensor_add(s.sacc[:], s.ep[:, 0], s.ep[:, 1])
            elif c >= 2:
                nc.vector.tensor_add(s.sacc[:], s.sacc[:], s.ep[:, c])

    def emit_ssum(s):
        """S = sum_c e[c] via PE identity-matmuls into the PSUM ring."""
        s.pS = s.psum.tile([P, NCH, 512], F32, tag=f"ps{s.b}")
        for c in range(C):
            for j in range(NCH):
                nc.tensor.matmul(
                    s.pS[:, j, 0:W],
                    lhsT=ident[:],
                    rhs=s.ep[:, c, j, :],
                    start=(c == 0),
                    stop=(c == C - 1),
                )

    def emit_recip(s):
        nc.vector.reciprocal_approx_fast(out=s.rr[:], in_=s.pS[:, :, 0:W])
        nc.vector.tensor_copy(s.rb[:], s.rr[:])

    def emit_pmul(s, c):
        eng = nc.gpsimd if c >= C - PMUL_POOL_CH else nc.vector
        eng.tensor_mul(out=s.ep[:, c], in0=s.ep[:, c], in1=s.rb[:])

    def emit_sub_dma(s, c):
        ot = outp.tile([P, NCH, W], BF16, tag="ot")
        eng = nc.gpsimd if c >= C - SUB_POOL_CH else nc.vector
        eng.tensor_sub(ot[:], s.x0[:, c], s.lnb[:])
        nc.sync.dma_start(
            out=out_d[s.b, c].rearrange("(j p) w -> p j w", p=P),
            in_=ot[:],
        )

    # --- build per-sample chunk schedules; each chunk maps pipeline-stage
    # -> emitter. The driver staggers sample B by DESYNC chunks and emits
    # stage-by-stage across samples so each engine's in-order stream has
    # the other sample's ready work ahead of any stalling op. ---
    def build_chunks(s):
        from functools import partial

        def pro_exp(s, c):
            # two channels per op: amortizes the ACT pipe/decode overhead
            nc.scalar.activation(
                out=s.ep[:, c : c + 2], in_=s.x0[:, c : c + 2], func=AF.Exp
            )

        def pmul_hconv(s, c):
            emit_pmul(s, c)
            emit_hconv(s, c)

        def ln_final(s):
            nc.scalar.activation(out=s.lnb[:], in_=s.sacc[:], func=AF.Ln)

        chunks = []
        for c in range(0, C, 2):
            chunks.append({3: partial(pro_exp, s, c)})
        chunks.append({0: partial(emit_ssum, s)})
        chunks.append({0: partial(emit_recip, s)})
        for it in range(n_iter):
            last = it == n_iter - 1
            for c in range(C):
                chunks.append({
                    0: partial(pmul_hconv, s, c),
                    1: partial(emit_o1, s, c),
                    2: partial(emit_wconv, s, c),
                    3: partial(emit_exp_psum, s, c, last),
                })
            if not last:
                chunks.append({0: partial(emit_ssum, s)})
                chunks.append({0: partial(emit_recip, s)})
            else:
                chunks.append({0: partial(ln_final, s)})
        for c in range(C):
            chunks.append({1: partial(emit_sub_dma, s, c)})
        return chunks

    scheds = [(build_chunks(s), b * DESYNC) for s, b in zip(S, range(n_samples))]
    n_slots = max(len(ch) + off for ch, off in scheds)
    for t in range(n_slots):
        for stage in range(4):
            for ch, off in scheds:
                i = t - off
                if 0 <= i < len(ch) and stage in ch[i]:
                    ch[i][stage]()


def build_nc(n_samples=BPC, n_iter=N_ITER, full_j0=False):
    nc = bacc.Bacc()
    x_in = nc.dram_tensor("x", [n_samples, C, H, W], BF16, kind="ExternalInput")
    th_in = nc.dram_tensor("th", [n_samples, H, H], BF16, kind="ExternalInput")
    tw_in = nc.dram_tensor("tw", [n_samples, W, W], BF16, kind="ExternalInput")
    id_in = nc.dram_tensor("ident", [P, P], BF16, kind="ExternalInput")
    out_d = nc.dram_tensor("out", [n_samples, C, H, W], BF16, kind="ExternalOutput")
    with tile.TileContext(nc) as tc:
        with ExitStack() as ctx:
            _crf_kernel(ctx, tc, out_d, x_in, th_in, tw_in, id_in, n_samples, n_iter)
    nc.finalize()
    return nc


def make_toeplitz(spacing, inv_theta, size, weight=1.0):
    d = spacing * np.arange(-(FS // 2), FS // 2 + 1, dtype=np.float32)
    k = np.exp(-((d * inv_theta) ** 2) / 2.0).astype(np.float32)
    k[FS // 2] = 0.0
    t = np.zeros((size, size), dtype=np.float32)
    for tap in range(FS):
        off = tap - FS // 2
        idx = np.arange(max(0, -off), min(size, size - off))
        t[idx + off, idx] = k[tap]
    return (t * weight).astype(np.float32)


def to_bf16(a):
    import ml_dtypes

    return np.asarray(a, dtype=np.float32).astype(ml_dtypes.bfloat16)


def host_prep(x, spatial_spacings, smoothness_weight, inv_smoothness_theta):
    w = float(np.asarray(smoothness_weight))
    th = np.stack(
        [
            make_toeplitz(float(spatial_spacings[b, 0]), float(inv_smoothness_theta[0]), H)
            for b in range(x.shape[0])
        ]
    )
    tw = np.stack(
        [
            make_toeplitz(
                float(spatial_spacings[b, 1]), float(inv_smoothness_theta[1]), W, weight=w
            )
            for b in range(x.shape[0])
        ]
    )
    return to_bf16(th), to_bf16(tw)


_NC_CACHE = {}


def kernel(x, spatial_spacings, smoothness_weight, inv_smoothness_theta):
    from concourse.bass_utils import run_bass_kernel_spmd

    x = to_bf16(x)
    spatial_spacings = np.asarray(spatial_spacings, dtype=np.float32)
    th, tw = host_prep(x, spatial_spacings, smoothness_weight, inv_smoothness_theta)
    ident = to_bf16(np.eye(P, dtype=np.float32))

    key = (BPC, N_ITER)
    if key not in _NC_CACHE:
        _NC_CACHE[key] = build_nc(BPC, N_ITER)
    nc = _NC_CACHE[key]

    core_ids = list(range(N_CORES))
    in_maps = []
    for i in core_ids:
        sl = slice(i * BPC, (i + 1) * BPC)
        in_maps.append({"x": x[sl], "th": th[sl], "tw": tw[sl], "ident": ident})
    res = run_bass_kernel_spmd(nc, in_maps, core_ids)
    out = np.concatenate([res.results[i]["out"] for i in core_ids], axis=0)
    return out.astype(np.float32)


if __name__ == "__main__":
    rng = np.random.default_rng(0)
    x = rng.standard_normal((B, C, H, W), dtype=np.float32)
    out = kernel(
        x,
        np.ones((B, 2), np.float32),
        np.float32(1.0),
        np.ones((2,), np.float32),
    )
    print(out.shape, out.dtype)
